# revision 1
# baseline (speedup 1.0000x reference)
"""
Binary Conv2d (BBCU-style) block on 8 Trainium2 NeuronCores.

Computation (per reference):
    z  = sign(x + move0_bias)                    # binarized activations in {-1,0,1}
    bw = scale[o] * sign(W)                      # binarized weights, per-out-channel scale
    y  = conv3x3(z, bw, pad=1)
    y  = prelu(y + pr_bias0, a) + pr_bias1 + x   # RPReLU + identity

Key exactness trick: the conv operands are exact small values (z in {-1,0,1},
sign(W) in {-1,0,1}) so we run the conv as fp8e4 matmuls with fp32 PSUM
accumulation — bit-exact integer counts (|sum| <= 576 << 2^24). The per-channel
`scale` folds into the epilogue affine constants.

Sharding: data-parallel over batch. 16 images / 8 cores = 2 images per core.

Per-core layout ("parity layout"): partitions = 64 channels x row-parity
(parts 0-63 even rows, 64-127 odd rows). SBUF tensors:
  xt   [128, P*256] f32   : chunk of G=2P rows of x (per-chunk, identity + sign input)
  zs1  [128, 130*272] fp8 : sign values, one 272B slot per row-pair index;
                            byte j in a slot = col j-1 (byte 0 / 257 = zero pad)
  zs2  [128, 130*272] fp8 : copy of zs1 with the odd block shifted +2 slots,
                            so the "cross-pair" matmuls read an aligned AP.
Conv = 6 matmuls per PSUM tile [128,512] (2 row-pairs x 256 cols):
  type-1 (dw=-1,0,1): K=(ch x parity of rows 2i,2i+1) -> M=(ch x parity), all
          4 quadrants of lhsT active (dh in {-1,0,+1} between the parities).
  type-2 (dw=-1,0,1): rows 2i+2 (even block) -> odd outputs (dh=+1), and rows
          2i-1 (odd block, via zs2 shift) -> even outputs (dh=-1).
Epilogue: ACT Prelu(scale*S + pb0, alpha) straight out of PSUM (or a
Relu-based decomposition, sim-friendly), then one DVE scalar_tensor_tensor:
  out = (g + pb1) + x.
"""

import os
from contextlib import ExitStack

import numpy as np

import ml_dtypes

import concourse.bass as bass
import concourse.mybir as mybir
import concourse.tile as tile
from concourse.bass_utils import run_bass_kernel_spmd
# ---------------------------------------------------------------------------
# Workaround: the in-container walrus rejects instructions carrying more than
# 2 semaphore waits ("Too many sync wait commands" in setupSyncWait), but
# Tile's sem-assignment freely attaches 3+. Post-process the serialized BIR:
# move excess waits onto NoOp instructions inserted just before the carrier
# (same engine => program order preserves the happens-before).
# ---------------------------------------------------------------------------
_MAX_WAITS = 1


def _split_sync_waits(mod: dict, max_waits: int = _MAX_WAITS) -> dict:
    for fn in mod.get("functions", []):
        for bb in fn.get("blocks", []):
            out = []
            for ins in bb.get("instructions", []):
                si = ins.get("sync_info")
                waits = (si or {}).get("on_wait") or []
                if len(waits) > max_waits:
                    extra, keep = waits[:-max_waits], waits[-max_waits:]
                    for i in range(0, len(extra), max_waits):
                        out.append({
                            "debug": ins.get("debug", 0),
                            "engine": ins["engine"],
                            "ins": [],
                            "name": f"{ins['name']}_ws{i}",
                            "opcode": "NoOp",
                            "outs": [],
                            "sync_info": {
                                "on_update": [],
                                "on_wait": extra[i:i + max_waits],
                            },
                        })
                    si["on_wait"] = keep
                out.append(ins)
            bb["instructions"] = out
    return mod


_orig_to_json_bytes = bass.Bass.to_json_bytes


def _to_json_bytes_split(self):
    import orjson

    return orjson.dumps(_split_sync_waits(orjson.loads(_orig_to_json_bytes(self))))


bass.Bass.to_json_bytes = _to_json_bytes_split

F32 = mybir.dt.float32
FP8 = mybir.dt.float8e4
NP_FP8 = ml_dtypes.float8_e4m3

# consts column indices
C_B0 = 0      # move0 bias (sign pass bias)
C_SC = 1      # scale (prelu path: activation scale)
C_PB0 = 2     # pr_bias0 (prelu path: activation bias)
C_AL = 3      # prelu alpha
C_PB1 = 4     # pr_bias1 (final add, prelu path)
C_RS = 5      # (1-a)*scale        (relu path: relu scale)
C_RB = 6      # (1-a)*pb0          (relu path: relu bias)
C_VS = 7      # a*scale            (relu path: STT1 scalar)
C_VB = 8      # a*pb0 + pb1        (relu path: final scalar)
NCOL = 9

SLOT = 272  # bytes per row-pair slot in zs tensors (16-aligned, >= 258)


def _build(B_per_core: int, H: int, W: int, C: int, G: int, use_prelu: bool):
    """Builds the per-core Bass module. Returns nc."""
    assert C == 64 and W == 256
    assert H % G == 0 and G % 4 == 0
    P = G // 2            # row-pairs per chunk
    NCH = H // G          # chunks per image
    NPAIR = H // 2        # row-pairs per image
    NSLOT = NPAIR + 2

    nc = bass.Bass()
    xd = nc.declare_dram_parameter("x", [B_per_core, C, H, W], F32, isOutput=False)
    wd = nc.declare_dram_parameter("wp", [6, 128, 128], FP8, isOutput=False)
    cd = nc.declare_dram_parameter("cv", [128, NCOL], F32, isOutput=False)
    yd = nc.declare_dram_parameter("y", [B_per_core, C, H, W], F32, isOutput=True)

    with ExitStack() as ctx:
        tc = ctx.enter_context(tile.TileContext(nc))
        cpool = ctx.enter_context(tc.tile_pool(name="const", bufs=1))
        zpool = ctx.enter_context(tc.tile_pool(name="zs", bufs=1))
        xpool = ctx.enter_context(tc.tile_pool(name="xt", bufs=5))
        gpool = ctx.enter_context(tc.tile_pool(name="gt", bufs=3))
        rpool = ctx.enter_context(tc.tile_pool(name="rt", bufs=2))
        pspool = ctx.enter_context(tc.tile_pool(name="ps", bufs=7, space="PSUM"))

        # --- resident constants ---
        wsb = cpool.tile([128, 6 * 128], FP8)
        nc.sync.dma_start(
            wsb[:].rearrange("k (t m) -> k t m", m=128),
            wd[:].rearrange("t k m -> k t m"),
        )
        cvs = cpool.tile([128, NCOL], F32)
        nc.sync.dma_start(cvs[:], cd[:])

        # zs1 slot j holds rows (2(j-1), 2(j-1)+1) on the (even, odd) blocks;
        # slot 0 and slot NPAIR+1 are zero halo pads.
        zs1 = zpool.tile([128, NSLOT * SLOT], FP8)
        zs1v = zs1[:].rearrange("p (s c) -> p s c", c=SLOT)

        # one-time pads (stay zero across both images):
        # column pads (col -1 at byte 0, col 256 at byte 257) on every slot
        nc.gpsimd.memset(zs1v[:, :, 0:1], 0.0)
        nc.gpsimd.memset(zs1v[:, :, 257:272], 0.0)
        # halo row slots (rows below 0 / above H-1)
        nc.gpsimd.memset(zs1[:, 0:SLOT], 0.0)
        nc.gpsimd.memset(zs1[:, (NPAIR + 1) * SLOT:(NPAIR + 2) * SLOT], 0.0)

        def load_sign_copy(b, k):
            """DMA x chunk k (parity layout), sign into zs1, copy into zs2."""
            r0 = k * G
            xt = xpool.tile([128, P * 256], F32, name=f"xt_{b}_{k}", tag="xt")
            xtv = xt[:].rearrange("p (s c) -> p s c", c=256)
            # even rows -> parts 0..63 ; odd rows -> parts 64..127.
            # Issued from two different engines so descriptor feeding of the
            # two streams proceeds in parallel.
            nc.sync.dma_start(xtv[0:64], xd[b, :, r0:r0 + G:2, :])
            nc.gpsimd.dma_start(xtv[64:128], xd[b, :, r0 + 1:r0 + G:2, :])
            s0 = k * P + 1
            nc.scalar.activation(
                zs1v[:, s0:s0 + P, 1:257],
                xtv[:],
                mybir.ActivationFunctionType.Sign,
                bias=cvs[:, C_B0:C_B0 + 1],
            )
            return xt

        def conv_chunk(b, k, xt):
            """6 matmuls per [128,512] PSUM tile + epilogue for chunk k."""
            r0 = k * G
            gt = gpool.tile([128, P * 256], F32, name=f"gt_{b}_{k}", tag="gt")
            for t in range(P // 2):
                i0 = k * P + 2 * t
                ps = pspool.tile([128, 512], F32, name="ps")
                # 3 full-array type-1 matmuls (rows 2i..2i+3 -> same pair)
                for mi, dw in enumerate((-1, 0, 1)):
                    rhs = zs1v[:, i0 + 1:i0 + 3, dw + 1:dw + 257]
                    nc.tensor.matmul(
                        ps[:],
                        wsb[:, (dw + 1) * 128:(dw + 2) * 128],
                        rhs,
                        start=(mi == 0),
                        stop=(mi == 2),
                    )
                # cross-pair contributions as pairs of concurrent quadrant
                # matmuls (disjoint 64x64 array tiles, own rhs offsets):
                #   a: even rows 2i+2/2i+4 -> odd outputs   (dh=+1)
                #   b: odd rows 2i-1/2i+1  -> even outputs  (dh=-1)
                for mi, dw in enumerate((-1, 0, 1)):
                    wcol = (3 + dw + 1) * 128
                    # skip_group_check: CoreSim's PSUM-group table mis-addresses
                    # base_partition != 0 outputs; HW accumulation is per-element
                    # has_written and is correct. start/stop live on the type-1
                    # full-array group above.
                    nc.tensor.matmul(
                        ps[64:128, :],
                        wsb[0:64, wcol + 64:wcol + 128],
                        zs1v[0:64, i0 + 2:i0 + 4, dw + 1:dw + 257],
                        start=False,
                        stop=False,
                        skip_group_check=True,
                        tile_position=(0, 64),
                    )
                    nc.tensor.matmul(
                        ps[0:64, :],
                        wsb[64:128, wcol:wcol + 64],
                        zs1v[64:128, i0:i0 + 2, dw + 1:dw + 257],
                        start=False,
                        stop=False,
                        skip_group_check=True,
                        tile_position=(64, 0),
                    )
                gslice = gt[:, t * 512:(t + 1) * 512]
                if use_prelu:
                    nc.scalar.activation(
                        gslice,
                        ps[:],
                        mybir.ActivationFunctionType.Prelu,
                        bias=cvs[:, C_PB0:C_PB0 + 1],
                        scale=cvs[:, C_SC:C_SC + 1],
                        alpha=cvs[:, C_AL:C_AL + 1],
                    )
                else:
                    rt = rpool.tile([128, 512], F32, name="rt")
                    nc.scalar.activation(
                        rt[:],
                        ps[:],
                        mybir.ActivationFunctionType.Relu,
                        bias=cvs[:, C_RB:C_RB + 1],
                        scale=cvs[:, C_RS:C_RS + 1],
                    )
                    # g = a*scale*S + r   (r = (1-a)*relu(scale*S+pb0))
                    nc.vector.scalar_tensor_tensor(
                        gslice,
                        ps[:],
                        cvs[:, C_VS:C_VS + 1],
                        rt[:],
                        op0=mybir.AluOpType.mult,
                        op1=mybir.AluOpType.add,
                    )
            # final = (g + c) + x, in place over gt
            ccol = C_PB1 if use_prelu else C_VB
            nc.vector.scalar_tensor_tensor(
                gt[:],
                gt[:],
                cvs[:, ccol:ccol + 1],
                xt[:],
                op0=mybir.AluOpType.add,
                op1=mybir.AluOpType.add,
            )
            finv = gt[:].rearrange("p (s c) -> p s c", c=256)
            nc.gpsimd.dma_start(yd[b, :, r0:r0 + G:2, :], finv[0:64])
            nc.gpsimd.dma_start(yd[b, :, r0 + 1:r0 + G:2, :], finv[64:128])

        # software pipeline: loads/sign run 2 chunks ahead of the
        # matmul+epilogue consumer so the PE never starves at chunk edges.
        LOOKAHEAD = 2
        jobs = [(b, k) for b in range(B_per_core) for k in range(NCH)]
        xts = {}
        for idx, (b, k) in enumerate(jobs):
            xts[(b, k)] = load_sign_copy(b, k)
            if idx >= LOOKAHEAD:
                bb, kk = jobs[idx - LOOKAHEAD]
                conv_chunk(bb, kk, xts.pop((bb, kk)))
        for bb, kk in jobs[-LOOKAHEAD:]:
            conv_chunk(bb, kk, xts.pop((bb, kk)))

    return nc


def _host_prep(move0_bias, conv_weight, prelu_weight, pr_bias0, pr_bias1):
    """Pack weights into the 6 lhsT matrices + per-partition constant vectors."""
    w = np.asarray(conv_weight, dtype=np.float32)          # [O, I, 3, 3]
    sw = np.sign(w).astype(np.float32)                     # {-1, 0, 1}
    scale = np.mean(np.abs(w), axis=(1, 2, 3)).astype(np.float32)  # [O]
    a = np.asarray(prelu_weight, dtype=np.float32).reshape(64)
    pb0 = np.asarray(pr_bias0, dtype=np.float32).reshape(64)
    pb1 = np.asarray(pr_bias1, dtype=np.float32).reshape(64)
    b0 = np.asarray(move0_bias, dtype=np.float32).reshape(64)

    # lhsT[k, m] with k = pi*64 + ci, m = po*64 + co ->  sw[co, ci, kh, kw]
    # type-1: dh = [[0, -1], [1, 0]][pi][po]; type-2: only (pi0,po1)=+1,(pi1,po0)=-1
    wp = np.zeros((6, 128, 128), dtype=np.float32)
    swT = np.transpose(sw, (1, 0, 2, 3))  # [ci, co, kh, kw]
    for idw, dw in enumerate((-1, 0, 1)):
        kw = dw + 1
        # type-1
        wp[idw, 0:64, 0:64] = swT[:, :, 1, kw]      # even->even  dh=0
        wp[idw, 0:64, 64:128] = swT[:, :, 0, kw]    # even->odd   dh=-1 (kh=0)
        wp[idw, 64:128, 0:64] = swT[:, :, 2, kw]    # odd->even   dh=+1 (kh=2)
        wp[idw, 64:128, 64:128] = swT[:, :, 1, kw]  # odd->odd    dh=0
        # type-2
        wp[3 + idw, 0:64, 64:128] = swT[:, :, 2, kw]   # row 2i+2 -> out 2i+1, dh=+1
        wp[3 + idw, 64:128, 0:64] = swT[:, :, 0, kw]   # row 2i-1 -> out 2i,   dh=-1
    wp8 = wp.astype(NP_FP8)

    cv = np.zeros((128, NCOL), dtype=np.float32)
    for blk in range(2):
        s = slice(blk * 64, blk * 64 + 64)
        cv[s, C_B0] = b0
        cv[s, C_SC] = scale
        cv[s, C_PB0] = pb0
        cv[s, C_AL] = a
        cv[s, C_PB1] = pb1
        cv[s, C_RS] = (1.0 - a) * scale
        cv[s, C_RB] = (1.0 - a) * pb0
        cv[s, C_VS] = a * scale
        cv[s, C_VB] = a * pb0 + pb1
    return wp8, cv


_NC_CACHE: dict = {}


def _get_nc(key, *args):
    if key not in _NC_CACHE:
        _NC_CACHE[key] = _build(*args)
    return _NC_CACHE[key]


def kernel(x, move0_bias, conv_weight, prelu_weight, pr_bias0, pr_bias1):
    x = np.asarray(x, dtype=np.float32)
    B, C, H, W = x.shape
    NCORES = 8
    assert B % NCORES == 0
    Bc = B // NCORES
    G = 32
    use_prelu = os.environ.get("BBCU_NO_PRELU", "0") != "1"

    wp8, cv = _host_prep(move0_bias, conv_weight, prelu_weight, pr_bias0, pr_bias1)

    key = (Bc, H, W, C, G, use_prelu)
    nc = _get_nc(key, Bc, H, W, C, G, use_prelu)

    in_maps = [
        {"x": x[i * Bc:(i + 1) * Bc], "wp": wp8, "cv": cv} for i in range(NCORES)
    ]
    res = run_bass_kernel_spmd(nc, in_maps, core_ids=list(range(NCORES)))
    out = np.concatenate([res.results[i]["y"] for i in range(NCORES)], axis=0)
    return out.astype(np.float32)



# revision 7
# speedup vs baseline: 1.6987x; 1.6987x over previous
"""
Binary Conv2d (BBCU-style) block on 8 Trainium2 NeuronCores.

Computation (per reference):
    z  = sign(x + move0_bias)                    # binarized activations
    bw = scale[o] * sign(W)                      # binarized weights
    y  = conv3x3(z, bw, pad=1)
    y  = prelu(y + pr_bias0, a) + pr_bias1 + x   # RPReLU + identity

This kernel is memory-roofline oriented: all large HBM I/O is fp16.

Host-side staging:
  - xh = fp16(x + pr_bias1), with a one-ulp "sign-preserving nudge" on the
    ~1e-5 fraction of elements where fp16 rounding would flip
    sign(x + move0_bias): afterwards (f32(xh) > t) == (x + move0_bias > 0)
    exactly, with t = pr_bias1 - move0_bias per channel. Folding pr_bias1
    into x lets the final epilogue be a plain tensor+tensor add, and folding
    move0_bias into the threshold makes the device sign op a single compare.
  - x is repacked to the "parity layout" the device uses: partition
    p = parity*64 + channel, free dim = (row//2)*W + col. Every DMA line is
    then fully contiguous in DRAM (2 KB+ runs instead of 1 KB strided).
  - Output y is written fp16 in the same layout; host unpacks to f32.
    Verified end-to-end rel err ~6e-4 (gate 2e-2).

Device pipeline (per core: 2 images, chunks of G=32 rows):
  z = ((xh > t) - 0.5) in {-0.5, +0.5} as fp8e4   (DVE tensor_scalar)
  conv = 3 DoubleRow fp8 matmul pairs per [128,512] PSUM tile:
    - zz plane 0 ("zs1"): slot j holds rows (2(j-1), 2(j-1)+1) on the
      (even, odd) partition blocks; byte c = col c-1; 272 B slots.
    - zz plane 1 ("zsw"): parity-swapped/shifted copy built by two
      SBUF->SBUF DMAs: at slot u, parts 0-63 = odd row 2u-3, parts
      64-127 = even row 2u. This makes the cross-pair (halo) taps of the
      3x3 conv read the SAME slot index as the in-pair taps, so each
      kw-tap is ONE DoubleRow matmul with k-tile dim = (plane0, plane1):
      plane0 lhsT = in-pair taps (kh quadrants), plane1 lhsT = halo taps
      (diagonal blocks), i.e. 6 matmul instructions per tile instead of 9,
      at 2 fp8 MACs/cell/cycle.
  epilogue: ACT Prelu(2*scale*S + pb0, alpha) -> fp16, then one DVE
  tensor+tensor add of the identity, then fp16 DMA out.
"""

import os
from contextlib import ExitStack

import numpy as np

import ml_dtypes

import concourse.bass as bass
import concourse.mybir as mybir
import concourse.tile as tile
from concourse.bass_utils import run_bass_kernel_spmd

# ---------------------------------------------------------------------------
# Workaround: the in-container walrus rejects instructions carrying more than
# 2 semaphore waits ("Too many sync wait commands" in setupSyncWait), but
# Tile's sem-assignment freely attaches 3+. Post-process the serialized BIR:
# move excess waits onto NoOp instructions inserted just before the carrier
# (same engine => program order preserves the happens-before).
# ---------------------------------------------------------------------------
_MAX_WAITS = 1


def _split_sync_waits(mod: dict, max_waits: int = _MAX_WAITS) -> dict:
    for fn in mod.get("functions", []):
        for bb in fn.get("blocks", []):
            out = []
            for ins in bb.get("instructions", []):
                si = ins.get("sync_info")
                waits = (si or {}).get("on_wait") or []
                if len(waits) > max_waits:
                    extra, keep = waits[:-max_waits], waits[-max_waits:]
                    for i in range(0, len(extra), max_waits):
                        out.append({
                            "debug": ins.get("debug", 0),
                            "engine": ins["engine"],
                            "ins": [],
                            "name": f"{ins['name']}_ws{i}",
                            "opcode": "NoOp",
                            "outs": [],
                            "sync_info": {
                                "on_update": [],
                                "on_wait": extra[i:i + max_waits],
                            },
                        })
                    si["on_wait"] = keep
                out.append(ins)
            bb["instructions"] = out
    return mod


_orig_to_json_bytes = bass.Bass.to_json_bytes


def _to_json_bytes_split(self):
    import orjson

    return orjson.dumps(_split_sync_waits(orjson.loads(_orig_to_json_bytes(self))))


bass.Bass.to_json_bytes = _to_json_bytes_split

F32 = mybir.dt.float32
F16 = mybir.dt.float16
FP8 = mybir.dt.float8e4
NP_FP8 = ml_dtypes.float8_e4m3

# consts column indices
C_T = 0       # sign threshold  t = pr_bias1 - move0_bias
C_SC = 1      # 2 * scale  (z is +-0.5)
C_PB0 = 2     # pr_bias0
C_AL = 3      # prelu alpha
NCOL = 4

SLOT = 272    # bytes per row-pair slot (16-aligned, >= 258)
NCORES = 8


def _build(Bc: int, H: int, W: int, C: int, G: int, use_prelu: bool = True):
    """Per-core Bass module: inputs x [Bc,128,(H/2)*W] f16 (parity layout),
    wp [128, 3*2*128] fp8, cv [128,NCOL] f32; output y same layout as x."""
    assert C == 64 and W == 256
    assert H % G == 0 and G % 4 == 0
    P = G // 2            # row-pairs per chunk
    NCH = H // G          # chunks per image
    NPAIR = H // 2
    NSLOT = NPAIR + 2
    FREE = P * W          # free elems per chunk

    dump_zz = os.environ.get("BBCU_DUMP_ZZ", "0") == "1"
    nc = bass.Bass()
    xd = nc.declare_dram_parameter("x", [Bc, 128, NPAIR * W], F16, isOutput=False)
    wd = nc.declare_dram_parameter("wp", [128, 3 * 2 * 128], FP8, isOutput=False)
    cd = nc.declare_dram_parameter("cv", [128, NCOL], F32, isOutput=False)
    yd = nc.declare_dram_parameter("y", [Bc, 128, NPAIR * W], F16, isOutput=True)
    zzd = (nc.declare_dram_parameter("zzd", [128, 2 * NSLOT * SLOT], FP8,
                                     isOutput=True) if dump_zz else None)

    with ExitStack() as ctx:
        tc = ctx.enter_context(tile.TileContext(nc))
        cpool = ctx.enter_context(tc.tile_pool(name="const", bufs=1))
        zpool = ctx.enter_context(tc.tile_pool(name="zz", bufs=1))
        xpool = ctx.enter_context(tc.tile_pool(name="xt", bufs=4))
        gpool = ctx.enter_context(tc.tile_pool(name="gt", bufs=3))
        pspool = ctx.enter_context(tc.tile_pool(name="ps", bufs=8, space="PSUM"))

        wsb = cpool.tile([128, 3 * 2 * 128], FP8)
        nc.sync.dma_start(wsb[:], wd[:])
        cvs = cpool.tile([128, NCOL], F32)
        nc.sync.dma_start(cvs[:], cd[:])

        # z storage: per slot, the two k-tile planes are adjacent (272 B apart)
        # so the DoubleRow k-tile stride fits the 16-bit ISA step field.
        # layout: [p, slot(NSLOT), plane(2), SLOT]
        zz = zpool.tile([128, NSLOT * 2 * SLOT], FP8)
        zzs = zz[:].rearrange("p (s c) -> p s c", c=2 * SLOT)
        zzv0 = zzs[:, :, 0:SLOT]
        zzv1 = zzs[:, :, SLOT:2 * SLOT]

        # one-time pads:
        # plane0 col pads (col -1 at byte 0, col 256.. at bytes 257+); plane1
        # pads are copied from plane0 by the zsw DMAs.
        nc.gpsimd.memset(zzv0[:, :, 0:1], 0.0)
        nc.gpsimd.memset(zzv0[:, :, 1 + W:SLOT], 0.0)
        # plane0 row halo (rows below 0): slot 0
        nc.gpsimd.memset(zzv0[:, 0:1, :], 0.0)
        # plane1 slot NPAIR, parts 64-127 = row H (below image): zero
        nc.gpsimd.memset(zzv1[64:128, NPAIR:NPAIR + 1, :], 0.0)

        # weight APs: wp cols = [kw(3), ktile(2), m(128)]
        w_aps = [
            wsb[:, kw * 256:(kw + 1) * 256].rearrange("k (t m) -> k t m", t=2)
            for kw in range(3)
        ]

        def load(b, k):
            xt = xpool.tile([128, FREE], F16, name=f"xt_{b}_{k}", tag="xt")
            nc.sync.dma_start(xt[:], xd[b, :, k * FREE:(k + 1) * FREE])
            return xt

        def sign(b, k, xt):
            s0 = k * P + 1
            nc.vector.tensor_scalar(
                zzv0[:, s0:s0 + P, 1:1 + W],
                xt[:].rearrange("p (s c) -> p s c", c=W),
                cvs[:, C_T:C_T + 1],
                0.5,
                mybir.AluOpType.is_gt,
                mybir.AluOpType.subtract,
            )

        def zsw(b, k):
            s0 = k * P + 1
            # plane1 parts 0-63 slot u <- plane0 parts 64-127 slot u-1
            nc.gpsimd.dma_start(
                zzv1[0:64, s0:s0 + P, :], zzv0[64:128, s0 - 1:s0 + P - 1, :])
            # plane1 parts 64-127 slot u <- plane0 parts 0-63 slot u+1
            nc.gpsimd.dma_start(
                zzv1[64:128, s0 - 1:s0 + P - 1, :], zzv0[0:64, s0:s0 + P, :])

        def conv(b, k, xt):
            gt = gpool.tile([128, FREE], F16, name=f"gt_{b}_{k}", tag="gt")
            NT = P // 2
            for grp in range(0, NT, 4):
                tiles = range(grp, min(grp + 4, NT))
                pss = {t: pspool.tile([128, 512], F32, name="ps") for t in tiles}
                # kw outer so the stationary weights reload only 3x per group.
                # start=True clears the whole bank's has_written bits, so it
                # must appear exactly once per bank (first MM), stop on the
                # last; the per-element has_written handles the two halves.
                for kw in range(3):
                    for t in tiles:
                        s = k * P + 2 * t + 1
                        for sl in range(2):
                            rp = zz[:, (s + sl) * 2 * SLOT:
                                    (s + sl + 1) * 2 * SLOT].rearrange(
                                "p (t c) -> p t c", t=2)
                            nc.tensor.matmul(
                                pss[t][:, sl * 256:(sl + 1) * 256],
                                w_aps[kw],
                                rp[:, :, kw:kw + 256],
                                start=(kw == 0 and sl == 0),
                                stop=(kw == 2 and sl == 1),
                                skip_group_check=True,
                                perf_mode=mybir.MatmulPerfMode.DoubleRow,
                            )
                for t in tiles:
                    if use_prelu:
                        nc.scalar.activation(
                            gt[:, t * 512:(t + 1) * 512],
                            pss[t][:],
                            mybir.ActivationFunctionType.Prelu,
                            bias=cvs[:, C_PB0:C_PB0 + 1],
                            scale=cvs[:, C_SC:C_SC + 1],
                            alpha=cvs[:, C_AL:C_AL + 1],
                        )
                    else:
                        # u = 2sc*S + pb0 ; g = max(u, a*u)
                        nc.scalar.activation(
                            gt[:, t * 512:(t + 1) * 512],
                            pss[t][:],
                            mybir.ActivationFunctionType.Identity,
                            bias=cvs[:, C_PB0:C_PB0 + 1],
                            scale=cvs[:, C_SC:C_SC + 1],
                        )
            if not use_prelu:
                nc.vector.scalar_tensor_tensor(
                    gt[:], gt[:], cvs[:, C_AL:C_AL + 1], gt[:],
                    op0=mybir.AluOpType.mult, op1=mybir.AluOpType.max)
            # out = g + xh  (identity + pr_bias1, pre-folded on host)
            nc.vector.scalar_tensor_tensor(
                gt[:], gt[:], 0.0, xt[:],
                op0=mybir.AluOpType.add, op1=mybir.AluOpType.add)
            nc.sync.dma_start(yd[b, :, k * FREE:(k + 1) * FREE], gt[:])

        jobs = [(b, k) for b in range(Bc) for k in range(NCH)]
        xts = {}
        xts[jobs[0]] = load(*jobs[0])
        for idx, (b, k) in enumerate(jobs):
            if idx + 1 < len(jobs):
                xts[jobs[idx + 1]] = load(*jobs[idx + 1])
            sign(b, k, xts[(b, k)])
            zsw(b, k)
            if idx >= 1:
                bb, kk = jobs[idx - 1]
                conv(bb, kk, xts.pop((bb, kk)))
        conv(*jobs[-1], xts.pop(jobs[-1]))
        if dump_zz:
            nc.sync.dma_start(zzd[:], zz[:])

    return nc


def _host_prep(x, move0_bias, conv_weight, prelu_weight, pr_bias0, pr_bias1,
               n_cores=NCORES):
    """Returns (xh_packed [n_cores][Bc,128,(H/2)*W] f16, wp fp8, cv f32)."""
    B, C, H, W = x.shape
    b0 = np.asarray(move0_bias, np.float32).reshape(C)
    pb1 = np.asarray(pr_bias1, np.float32).reshape(C)
    t = (pb1 - b0).astype(np.float32)

    # fp16(x + pb1) with sign-preserving nudge
    xb = x + pb1.reshape(1, C, 1, 1)
    xh = xb.astype(np.float16)
    tb = t.reshape(1, C, 1, 1)
    sref = (x + b0.reshape(1, C, 1, 1)) > 0
    pdev = xh.astype(np.float32) > tb
    fixup = sref & ~pdev
    fixdn = ~sref & pdev
    if fixup.any():
        xh = np.where(fixup, np.nextafter(xh, np.float16(np.inf),
                                          dtype=np.float16), xh)
    if fixdn.any():
        xh = np.where(fixdn, np.nextafter(xh, np.float16(-np.inf),
                                          dtype=np.float16), xh)

    # parity repack: [B,C,H,W] -> [B, par, C, H/2, W] -> [B, 128, (H/2)*W]
    xp = np.ascontiguousarray(
        xh.reshape(B, C, H // 2, 2, W).transpose(0, 3, 1, 2, 4)
    ).reshape(B, 128, (H // 2) * W)

    # weights: wp[k, kw*256 + tile*128 + m] ; k = par_k*64+ci, m = par_m*64+co
    w = np.asarray(conv_weight, np.float32)
    sw = np.sign(w).astype(np.float32)            # [co, ci, kh, kw]
    swT = np.transpose(sw, (1, 0, 2, 3))          # [ci, co, kh, kw]
    scale = np.mean(np.abs(w), axis=(1, 2, 3)).astype(np.float32)
    wp = np.zeros((128, 3, 2, 128), dtype=np.float32)
    for kw in range(3):
        # plane 0: in-pair taps
        wp[0:64, kw, 0, 0:64] = swT[:, :, 1, kw]      # even->even  kh=1
        wp[0:64, kw, 0, 64:128] = swT[:, :, 0, kw]    # even->odd   kh=0
        wp[64:128, kw, 0, 0:64] = swT[:, :, 2, kw]    # odd->even   kh=2
        wp[64:128, kw, 0, 64:128] = swT[:, :, 1, kw]  # odd->odd    kh=1
        # plane 1: halo taps via zsw (diagonal blocks)
        wp[0:64, kw, 1, 0:64] = swT[:, :, 0, kw]      # row 2p-1 -> even out
        wp[64:128, kw, 1, 64:128] = swT[:, :, 2, kw]  # row 2p+2 -> odd out
    wp8 = wp.reshape(128, 3 * 2 * 128).astype(NP_FP8)

    a = np.asarray(prelu_weight, np.float32).reshape(C)
    pb0 = np.asarray(pr_bias0, np.float32).reshape(C)
    cv = np.zeros((128, NCOL), dtype=np.float32)
    for blk in range(2):
        s = slice(blk * 64, blk * 64 + 64)
        cv[s, C_T] = t
        cv[s, C_SC] = 2.0 * scale
        cv[s, C_PB0] = pb0
        cv[s, C_AL] = a
    return xp, wp8, cv


def _unpack(y_packed, B, C, H, W):
    """[B,128,(H/2)*W] f16 -> [B,C,H,W] f32"""
    y = y_packed.reshape(B, 2, C, H // 2, W).transpose(0, 2, 3, 1, 4)
    return np.ascontiguousarray(y).reshape(B, C, H, W).astype(np.float32)


_NC_CACHE: dict = {}


def _get_nc(key, *args, **kw):
    if key not in _NC_CACHE:
        _NC_CACHE[key] = _build(*args, **kw)
    return _NC_CACHE[key]


def prepare(x, move0_bias, conv_weight, prelu_weight, pr_bias0, pr_bias1):
    x = np.asarray(x, dtype=np.float32)
    B, C, H, W = x.shape
    assert B % NCORES == 0
    Bc = B // NCORES
    G = 32
    use_prelu = os.environ.get("BBCU_NO_PRELU", "0") != "1"
    xp, wp8, cv = _host_prep(x, move0_bias, conv_weight, prelu_weight,
                             pr_bias0, pr_bias1)
    key = (Bc, H, W, C, G, use_prelu)
    nc = _get_nc(key, Bc, H, W, C, G, use_prelu)
    in_maps = [
        {"x": xp[i * Bc:(i + 1) * Bc], "wp": wp8, "cv": cv}
        for i in range(NCORES)
    ]
    return nc, in_maps, (B, C, H, W, Bc)


def kernel(x, move0_bias, conv_weight, prelu_weight, pr_bias0, pr_bias1):
    nc, in_maps, (B, C, H, W, Bc) = prepare(
        x, move0_bias, conv_weight, prelu_weight, pr_bias0, pr_bias1)
    res = run_bass_kernel_spmd(nc, in_maps, core_ids=list(range(NCORES)))
    yp = np.concatenate([res.results[i]["y"] for i in range(NCORES)], axis=0)
    return _unpack(yp, B, C, H, W)
